# revision 5
# baseline (speedup 1.0000x reference)
"""JointEBM Langevin sampler, data-parallel across 8 NeuronCores.

Pure data parallel per the sharding hint: batch rows are split across the
8 devices, the small MLP weights are replicated.  Only grad_y is needed,
derived by hand:

    z1 = [x,y] @ W1 + b1 ; h1 = relu(z1)
    z2 = h1 @ W2 + b2    ; (h2 = relu(z2) never needed beyond its mask)
    g2 = W3[:, t]  (constant across steps; t fixed)
    gy = ((g2 * (z2>0)) @ W2.T * (z1>0)) @ W1y.T
    y <- y - LR * gy

x @ W1x + b1 is constant across the 20 steps and is computed once.
"""
import numpy as np

LR = 0.1
B, DX, DY, H, K = 65536, 256, 64, 512, 4
NCORES = 8

_compiled = None


def _build(steps):
    import jax
    import jax.numpy as jnp
    try:
        jax.config.update("jax_compilation_cache_dir", "/tmp/jax_ebm_cache")
        jax.config.update("jax_persistent_cache_min_compile_time_secs", 1.0)
    except Exception:
        pass

    def per_core(x, tcl, W1x, W1y, b1, W2, b2, W3):
        xc = x @ W1x + b1                      # [b, H] constant part of z1
        g2 = W3.T[tcl]                         # [b, H] rows = W3[:, t_b]
        W2T = W2.T
        W1yT = W1y.T

        def step(y, _):
            z1 = xc + y @ W1y
            h1 = jax.nn.relu(z1)
            z2 = h1 @ W2 + b2
            g2m = jnp.where(z2 > 0, g2, 0.0)
            g1 = g2m @ W2T
            g1m = jnp.where(z1 > 0, g1, 0.0)
            gy = g1m @ W1yT
            return y - LR * gy, None

        y0 = jnp.zeros((x.shape[0], DY), x.dtype)
        y, _ = jax.lax.scan(step, y0, None, length=steps)
        return y

    return jax.pmap(per_core, axis_name="i",
                    in_axes=(0, 0, None, None, None, None, None, None))


def kernel(x, t, W1, b1, W2, b2, W3, b3, steps):
    global _compiled
    import jax

    x = np.asarray(x, dtype=np.float32)
    t = np.asarray(t)
    W1 = np.asarray(W1, dtype=np.float32)
    b1 = np.asarray(b1, dtype=np.float32)
    W2 = np.asarray(W2, dtype=np.float32)
    b2 = np.asarray(b2, dtype=np.float32)
    W3 = np.asarray(W3, dtype=np.float32)
    steps = int(steps)

    n = x.shape[0]
    per = n // NCORES
    tc = np.clip(t, 0, None).astype(np.int32)

    xs = x.reshape(NCORES, per, DX)
    ts = tc.reshape(NCORES, per)

    W1x = np.ascontiguousarray(W1[:DX, :])
    W1y = np.ascontiguousarray(W1[DX:, :])

    if _compiled is None:
        _compiled = _build(steps)
    y = _compiled(xs, ts, W1x, W1y, b1, W2, b2, W3)
    y = np.asarray(jax.device_get(y)).reshape(n, DY).astype(np.float32)
    return y


if __name__ == "__main__":
    rng = np.random.default_rng(0)
    x = rng.standard_normal((B, DX), dtype=np.float32)
    t = rng.integers(0, K, size=(B,)).astype(np.int64)
    s1 = 1.0 / np.sqrt(DX + DY)
    s2 = 1.0 / np.sqrt(H)
    W1 = (rng.standard_normal((DX + DY, H)) * s1).astype(np.float32)
    W2 = (rng.standard_normal((H, H)) * s2).astype(np.float32)
    W3 = (rng.standard_normal((H, K)) * s2).astype(np.float32)
    out = kernel(x=x, t=t, W1=W1, b1=np.zeros(H, np.float32), W2=W2,
                 b2=np.zeros(H, np.float32), W3=W3,
                 b3=np.zeros(K, np.float32), steps=20)
    print(out.shape, out.dtype, np.abs(out).mean())
